# revision 49
# baseline (speedup 1.0000x reference)
"""Trainium2 Bass kernel for nn_DigitCap (sparse_attention).

Math note: the reference's softmax is over a size-1 axis, so C == 1 exactly
and the whole N x N attention matrix A is dead code.  The computation
collapses to

    S[b,d,i]  = sum_{n,j} (1 + B[d,n]) * W[d,n,i,j] * U[b,n,j]
    out[b,d,:] = (1 - exp(-|S|)) * S / (|S| + 1e-7)

which is a single 4096-deep contraction per (b, d*i) plus a tiny squash
epilogue.  Strategy: pure data parallel over batch (64 / 8 cores = 8 per
core); W and B replicated; (1+B) folded into W on device with one fused
scalar_tensor_tensor op per W group.

Written in raw Bass (explicit semaphores): the Tile framework's tail drain
emits more sem waits per instruction than this toolchain's codegen accepts.
"""

import numpy as np
from contextlib import ExitStack

import concourse.bass as bass
import concourse.mybir as mybir
from concourse.bass_utils import run_bass_kernel_spmd

F32 = mybir.dt.float32
AF = mybir.ActivationFunctionType
P = 128
D, DD, N, DP = 10, 16, 512, 8     # digit caps, digit dim, primary caps, primary dim
DI = D * DD                        # 160
K = N * DP                         # 4096 contraction
NCHUNK = K // P                    # 32 chunks of 128 contraction rows
NG = 8                             # DVE scale groups (4 chunks each)
GC = NCHUNK // NG                  # 4 chunks per group
WSIZES = [2, 2, 4, 4, 4, 4, 4, 4, 2, 2]  # chunks per W DMA (small first+last)
NWD = len(WSIZES)
NCORES = 8
BFULL = 64
BB = BFULL // NCORES               # 8 batches per core
EPS = 1e-7
NWARM = 0                          # PE HAM warm-up abandoned: dummies block the real matmuls


def build_raw():
    nc = bass.Bass()
    u_t = nc.dram_tensor("u_t", [P, NCHUNK * BB], F32, kind="ExternalInput")
    w_t = nc.dram_tensor("w_t", [P, NCHUNK * DI], F32, kind="ExternalInput")
    # host-permuted + replicated B: bp[j*16+n_l, c*D+d] = B[d, c*16+n_l]
    bp = nc.dram_tensor("bp", [P, NCHUNK * D], F32, kind="ExternalInput")
    out = nc.dram_tensor("out", [BB, DI], F32, kind="ExternalOutput")

    with ExitStack() as ctx:
        u_all = ctx.enter_context(nc.sbuf_tensor("u_all", [P, NCHUNK * BB], F32))
        w_all = ctx.enter_context(nc.sbuf_tensor("w_all", [P, NCHUNK * DI], F32))
        bsc = ctx.enter_context(nc.sbuf_tensor("bsc", [P, NCHUNK * D], F32))
        ps = ctx.enter_context(nc.psum_tensor("ps", [BB, DI], F32))
        psw = ctx.enter_context(nc.psum_tensor("psw", [BB, 320], F32))
        s = ctx.enter_context(nc.sbuf_tensor("s", [BB, DI], F32))
        sq = ctx.enter_context(nc.sbuf_tensor("sq", [BB, DI], F32))
        ss = ctx.enter_context(nc.sbuf_tensor("ss", [BB, D], F32))
        normt = ctx.enter_context(nc.sbuf_tensor("norm", [BB, D], F32))
        den = ctx.enter_context(nc.sbuf_tensor("den", [BB, D], F32))
        rec = ctx.enter_context(nc.sbuf_tensor("rec", [BB, D], F32))
        et = ctx.enter_context(nc.sbuf_tensor("et", [BB, D], F32))
        numt = ctx.enter_context(nc.sbuf_tensor("numt", [BB, D], F32))
        ft = ctx.enter_context(nc.sbuf_tensor("ft", [BB, D], F32))
        ot = ctx.enter_context(nc.sbuf_tensor("ot", [BB, DI], F32))
        warm = ctx.enter_context(nc.sbuf_tensor("warm", [1, 4], F32))
        sem_u = ctx.enter_context(nc.semaphore("sem_u"))
        sem_bc = ctx.enter_context(nc.semaphore("sem_bc"))
        sem_w = [ctx.enter_context(nc.semaphore(f"sem_w{g}")) for g in range(NWD)]
        sem_dve = ctx.enter_context(nc.semaphore("sem_dve"))
        sem_pe = ctx.enter_context(nc.semaphore("sem_pe"))
        sem_v2 = ctx.enter_context(nc.semaphore("sem_v2"))
        sem_act1 = ctx.enter_context(nc.semaphore("sem_act1"))
        sem_act2 = ctx.enter_context(nc.semaphore("sem_act2"))
        sem_fin = ctx.enter_context(nc.semaphore("sem_fin"))
        sem_out = ctx.enter_context(nc.semaphore("sem_out"))
        sem_wm = ctx.enter_context(nc.semaphore("sem_wm"))
        sem_s1 = ctx.enter_context(nc.semaphore("sem_s1"))
        # same-engine RAW chain sems (for the race detector; the waits are
        # satisfied by the time they execute, so they cost ~nothing)
        sem_c2 = ctx.enter_context(nc.semaphore("sem_c2"))
        sem_c3 = ctx.enter_context(nc.semaphore("sem_c3"))
        sem_c4 = ctx.enter_context(nc.semaphore("sem_c4"))
        sem_c5 = ctx.enter_context(nc.semaphore("sem_c5"))

        with nc.Block() as block:

            @block.sync
            def _(sync):
                # u + odd W transfers here; bp + even W transfers go out on
                # the ACT HWDGE ring in parallel.
                sync.dma_start(u_all[:], u_t[:, :]).then_inc(sem_u, 16)
                coff = 0
                for g, wsz in enumerate(WSIZES):
                    if g % 2 == 1:
                        sync.dma_start(
                            w_all[:, coff * DI:(coff + wsz) * DI],
                            bass.AP(
                                w_t, coff * DI,
                                [[NCHUNK * DI, P], [1, wsz * DI]],
                            ),
                        ).then_inc(sem_w[g], 16)
                    coff += wsz
                # output
                sync.wait_ge(sem_fin, 1)
                sync.dma_start(out[:, :], ot[:]).then_inc(sem_out, 16)
                sync.wait_ge(sem_out, 16)

            @block.gpsimd
            def _(gpsimd):
                # seed for the ACT table warm-up activations
                gpsimd.memset(warm[:], 1.0).then_inc(sem_wm, 1)

            @block.vector
            def _(vector):
                vector.wait_ge(sem_bc, 16)
                # fused (bsc + 1) * W, in place, one op per W transfer
                coff = 0
                for g, wsz in enumerate(WSIZES):
                    vector.wait_ge(sem_w[g], 16)
                    w_v = w_all[:, coff * DI:(coff + wsz) * DI].rearrange(
                        "p (c d i) -> p c d i", d=D, i=DD
                    )
                    vector.scalar_tensor_tensor(
                        out=w_v,
                        in0=bsc[:, coff * D:(coff + wsz) * D]
                        .rearrange("p (c d) -> p c d", d=D)
                        .broadcast_to([P, wsz, D, DD]),
                        scalar=1.0,
                        in1=w_v,
                        op0=mybir.AluOpType.add,
                        op1=mybir.AluOpType.mult,
                    ).then_inc(sem_dve, 1)
                    coff += wsz
                # epilogue part 1: squares + row sums (s copied by ACT)
                vector.wait_ge(sem_s1, 1)
                s3 = s[:].rearrange("b (d i) -> b d i", i=DD)
                vector.tensor_mul(
                    out=sq[:].rearrange("b (d i) -> b d i", i=DD), in0=s3, in1=s3
                ).then_inc(sem_c2, 1)
                vector.wait_ge(sem_c2, 1)
                vector.tensor_reduce(
                    out=ss[:], in_=sq[:].rearrange("b (d i) -> b d i", i=DD),
                    axis=mybir.AxisListType.X, op=mybir.AluOpType.add,
                ).then_inc(sem_v2, 1)
                # den/rec/o1 only need norm (Sqrt) -> they run while ACT
                # loads the Exp table
                vector.wait_ge(sem_act1, 1)
                vector.tensor_scalar_add(
                    out=den[:], in0=normt[:], scalar1=EPS
                ).then_inc(sem_c4, 1)
                vector.wait_ge(sem_c4, 1)
                vector.reciprocal(out=rec[:], in_=den[:]).then_inc(sem_c4, 1)
                vector.wait_ge(sem_c4, 2)
                vector.tensor_mul(
                    out=ot[:].rearrange("b (d i) -> b d i", i=DD),
                    in0=s3, in1=rec[:].broadcast_to([BB, D, DD]),
                ).then_inc(sem_c4, 1)
                vector.wait_ge(sem_act2, 1)
                vector.tensor_scalar(
                    out=numt[:], in0=et[:], scalar1=-1.0, scalar2=1.0,
                    op0=mybir.AluOpType.mult, op1=mybir.AluOpType.add,
                ).then_inc(sem_c4, 1)
                vector.wait_ge(sem_c4, 4)
                o3 = ot[:].rearrange("b (d i) -> b d i", i=DD)
                vector.tensor_mul(
                    out=o3, in0=o3, in1=numt[:].broadcast_to([BB, D, DD]),
                ).then_inc(sem_fin, 1)

            @block.tensor
            def _(tensor):
                tensor.wait_ge(sem_u, 16)
                coff = 0
                for g, wsz in enumerate(WSIZES):
                    tensor.wait_ge(sem_dve, g + 1)
                    for k in range(wsz):
                        c = coff + k
                        mm = tensor.matmul(
                            ps[:],
                            lhsT=u_all[:, c * BB:(c + 1) * BB],
                            rhs=w_all[:, c * DI:(c + 1) * DI],
                            start=(c == 0), stop=(c == NCHUNK - 1),
                        )
                    coff += wsz
                mm.then_inc(sem_pe, 1)

            @block.scalar
            def _(scalar):
                # bp + even W transfers on the ACT HWDGE ring
                scalar.dma_start(bsc[:], bp[:, :]).then_inc(sem_bc, 16)
                coff = 0
                for g, wsz in enumerate(WSIZES):
                    if g % 2 == 0:
                        scalar.dma_start(
                            w_all[:, coff * DI:(coff + wsz) * DI],
                            bass.AP(
                                w_t, coff * DI,
                                [[NCHUNK * DI, P], [1, wsz * DI]],
                            ),
                        ).then_inc(sem_w[g], 16)
                    coff += wsz
                # ACT table warm-up.  The table RAM holds ONE function and
                # Copy/Sqrt share a table, so warming Sqrt covers both; Exp's
                # load is hidden behind DVE work.
                scalar.wait_ge(sem_wm, 1)
                scalar.activation(out=warm[:, 0:1], in_=warm[:, 1:2], func=AF.Sqrt)
                # epilogue: S copy (Copy shares the warmed Sqrt table)
                scalar.wait_ge(sem_pe, 1)
                scalar.activation(out=s[:], in_=ps[:], func=AF.Copy).then_inc(
                    sem_s1, 1
                )
                scalar.wait_ge(sem_v2, 1)
                scalar.activation(out=normt[:], in_=ss[:], func=AF.Sqrt).then_inc(
                    sem_act1, 1
                )
                scalar.wait_ge(sem_act1, 1)
                scalar.activation(
                    out=et[:], in_=normt[:], func=AF.Exp, scale=-1.0
                ).then_inc(sem_act2, 1)

    return nc


_CACHE = {}


def _get_nc():
    if "nc" not in _CACHE:
        _CACHE["nc"] = build_raw()
    return _CACHE["nc"]


def prep_inputs(primary_caps, W, B):
    """Host-side layout prep + sharding (no arithmetic).

    Contraction row order: chunk c holds n in [c*16, (c+1)*16); within a
    chunk, partition p = j*16 + n_local.
    """
    U = np.asarray(primary_caps, dtype=np.float32)
    Wnj = np.transpose(np.asarray(W, dtype=np.float32), (1, 3, 0, 2))  # n j d i
    # [c, p=(j,n_l), f] -> [p, (c f)]: one contiguous run per partition/group
    Wt = np.ascontiguousarray(
        Wnj.reshape(NCHUNK, 16, DP, DI)
        .transpose(0, 2, 1, 3)
        .reshape(NCHUNK, P, DI)
        .transpose(1, 0, 2)
        .reshape(P, NCHUNK * DI)
    )
    # bp[j*16+n_l, c*D+d] = B[d, c*16+n_l]  (replicated 8x across j)
    bp16 = (
        np.asarray(B, dtype=np.float32)
        .reshape(D, NCHUNK, 16)
        .transpose(2, 1, 0)
        .reshape(1, 16, NCHUNK * D)
    )
    bpm = np.ascontiguousarray(
        np.broadcast_to(bp16, (DP, 16, NCHUNK * D)).reshape(P, NCHUNK * D)
    )
    Unj = np.transpose(U, (1, 2, 0))  # n j b
    Ut = (
        Unj.reshape(NCHUNK, 16, DP, BFULL)
        .transpose(0, 2, 1, 3)
        .reshape(NCHUNK, P, BFULL)
        .transpose(1, 0, 2)  # [p, c, b]
    )
    return [
        {
            "u_t": np.ascontiguousarray(
                Ut[:, :, c * BB:(c + 1) * BB].reshape(P, NCHUNK * BB)
            ),
            "w_t": Wt,
            "bp": bpm,
        }
        for c in range(NCORES)
    ]


def kernel(primary_caps, W, B):
    nc = _get_nc()
    in_maps = prep_inputs(primary_caps, W, B)
    res = run_bass_kernel_spmd(nc, in_maps, core_ids=list(range(NCORES)))
    outs = [res.results[c]["out"] for c in range(NCORES)]
    return np.concatenate(outs, axis=0).reshape(BFULL, D, DD).astype(np.float32)


# revision 50
# speedup vs baseline: 1.1206x; 1.1206x over previous
"""Trainium2 Bass kernel for nn_DigitCap (sparse_attention).

Math note: the reference's softmax is over a size-1 axis, so C == 1 exactly
and the whole N x N attention matrix A is dead code.  The computation
collapses to

    S[b,d,i]  = sum_{n,j} (1 + B[d,n]) * W[d,n,i,j] * U[b,n,j]
    out[b,d,:] = (1 - exp(-|S|)) * S / (|S| + 1e-7)

which is a single 4096-deep contraction per (b, d*i) plus a tiny squash
epilogue.  Strategy: pure data parallel over batch (64 / 8 cores = 8 per
core); W and B replicated; (1+B) folded into W on device with one fused
scalar_tensor_tensor op per W group.

Written in raw Bass (explicit semaphores): the Tile framework's tail drain
emits more sem waits per instruction than this toolchain's codegen accepts.
"""

import numpy as np
from contextlib import ExitStack

import concourse.bass as bass
import concourse.mybir as mybir
from concourse.bass_utils import run_bass_kernel_spmd

F32 = mybir.dt.float32
AF = mybir.ActivationFunctionType
P = 128
D, DD, N, DP = 10, 16, 512, 8     # digit caps, digit dim, primary caps, primary dim
DI = D * DD                        # 160
K = N * DP                         # 4096 contraction
NCHUNK = K // P                    # 32 chunks of 128 contraction rows
NG = 8                             # DVE scale groups (4 chunks each)
GC = NCHUNK // NG                  # 4 chunks per group
WSIZES = [4, 4, 4, 4, 4, 4, 4, 4]  # chunks per W DMA
NWD = len(WSIZES)
NCORES = 8
BFULL = 64
BB = BFULL // NCORES               # 8 batches per core
EPS = 1e-7
NWARM = 0                          # PE HAM warm-up abandoned: dummies block the real matmuls


def build_raw():
    nc = bass.Bass()
    u_t = nc.dram_tensor("u_t", [P, NCHUNK * BB], F32, kind="ExternalInput")
    w_t = nc.dram_tensor("w_t", [P, NCHUNK * DI], F32, kind="ExternalInput")
    # host-permuted + replicated B: bp[j*16+n_l, c*D+d] = B[d, c*16+n_l]
    bp = nc.dram_tensor("bp", [P, NCHUNK * D], F32, kind="ExternalInput")
    out = nc.dram_tensor("out", [BB, DI], F32, kind="ExternalOutput")

    with ExitStack() as ctx:
        u_all = ctx.enter_context(nc.sbuf_tensor("u_all", [P, NCHUNK * BB], F32))
        w_all = ctx.enter_context(nc.sbuf_tensor("w_all", [P, NCHUNK * DI], F32))
        bsc = ctx.enter_context(nc.sbuf_tensor("bsc", [P, NCHUNK * D], F32))
        ps = ctx.enter_context(nc.psum_tensor("ps", [BB, DI], F32))
        psw = ctx.enter_context(nc.psum_tensor("psw", [BB, 320], F32))
        s = ctx.enter_context(nc.sbuf_tensor("s", [BB, DI], F32))
        sq = ctx.enter_context(nc.sbuf_tensor("sq", [BB, DI], F32))
        ss = ctx.enter_context(nc.sbuf_tensor("ss", [BB, D], F32))
        normt = ctx.enter_context(nc.sbuf_tensor("norm", [BB, D], F32))
        den = ctx.enter_context(nc.sbuf_tensor("den", [BB, D], F32))
        rec = ctx.enter_context(nc.sbuf_tensor("rec", [BB, D], F32))
        et = ctx.enter_context(nc.sbuf_tensor("et", [BB, D], F32))
        numt = ctx.enter_context(nc.sbuf_tensor("numt", [BB, D], F32))
        ft = ctx.enter_context(nc.sbuf_tensor("ft", [BB, D], F32))
        ot = ctx.enter_context(nc.sbuf_tensor("ot", [BB, DI], F32))
        warm = ctx.enter_context(nc.sbuf_tensor("warm", [1, 4], F32))
        sem_u = ctx.enter_context(nc.semaphore("sem_u"))
        sem_bc = ctx.enter_context(nc.semaphore("sem_bc"))
        sem_w = [ctx.enter_context(nc.semaphore(f"sem_w{g}")) for g in range(NWD)]
        sem_dve = ctx.enter_context(nc.semaphore("sem_dve"))
        sem_pe = ctx.enter_context(nc.semaphore("sem_pe"))
        sem_v2 = ctx.enter_context(nc.semaphore("sem_v2"))
        sem_act1 = ctx.enter_context(nc.semaphore("sem_act1"))
        sem_act2 = ctx.enter_context(nc.semaphore("sem_act2"))
        sem_fin = ctx.enter_context(nc.semaphore("sem_fin"))
        sem_out = ctx.enter_context(nc.semaphore("sem_out"))
        sem_wm = ctx.enter_context(nc.semaphore("sem_wm"))
        sem_s1 = ctx.enter_context(nc.semaphore("sem_s1"))
        # same-engine RAW chain sems (for the race detector; the waits are
        # satisfied by the time they execute, so they cost ~nothing)
        sem_c2 = ctx.enter_context(nc.semaphore("sem_c2"))
        sem_c3 = ctx.enter_context(nc.semaphore("sem_c3"))
        sem_c4 = ctx.enter_context(nc.semaphore("sem_c4"))
        sem_c5 = ctx.enter_context(nc.semaphore("sem_c5"))

        with nc.Block() as block:

            @block.sync
            def _(sync):
                # bp first (tiny, gates the DVE), then all W transfers
                sync.dma_start(bsc[:], bp[:, :]).then_inc(sem_bc, 16)
                coff = 0
                for g, wsz in enumerate(WSIZES):
                    sync.dma_start(
                        w_all[:, coff * DI:(coff + wsz) * DI],
                        bass.AP(
                            w_t, coff * DI,
                            [[NCHUNK * DI, P], [1, wsz * DI]],
                        ),
                    ).then_inc(sem_w[g], 16)
                    coff += wsz
                # output
                sync.wait_ge(sem_fin, 1)
                sync.dma_start(out[:, :], ot[:]).then_inc(sem_out, 16)
                sync.wait_ge(sem_out, 16)

            @block.gpsimd
            def _(gpsimd):
                # seed for the ACT table warm-up activations
                gpsimd.memset(warm[:], 1.0).then_inc(sem_wm, 1)

            @block.vector
            def _(vector):
                vector.wait_ge(sem_bc, 16)
                # fused (bsc + 1) * W, in place, one op per W transfer
                coff = 0
                for g, wsz in enumerate(WSIZES):
                    vector.wait_ge(sem_w[g], 16)
                    w_v = w_all[:, coff * DI:(coff + wsz) * DI].rearrange(
                        "p (c d i) -> p c d i", d=D, i=DD
                    )
                    vector.scalar_tensor_tensor(
                        out=w_v,
                        in0=bsc[:, coff * D:(coff + wsz) * D]
                        .rearrange("p (c d) -> p c d", d=D)
                        .broadcast_to([P, wsz, D, DD]),
                        scalar=1.0,
                        in1=w_v,
                        op0=mybir.AluOpType.add,
                        op1=mybir.AluOpType.mult,
                    ).then_inc(sem_dve, 1)
                    coff += wsz
                # epilogue part 1: squares + row sums (s copied by ACT)
                vector.wait_ge(sem_s1, 1)
                s3 = s[:].rearrange("b (d i) -> b d i", i=DD)
                vector.tensor_mul(
                    out=sq[:].rearrange("b (d i) -> b d i", i=DD), in0=s3, in1=s3
                ).then_inc(sem_c2, 1)
                vector.wait_ge(sem_c2, 1)
                vector.tensor_reduce(
                    out=ss[:], in_=sq[:].rearrange("b (d i) -> b d i", i=DD),
                    axis=mybir.AxisListType.X, op=mybir.AluOpType.add,
                ).then_inc(sem_v2, 1)
                # den/rec/o1 only need norm (Sqrt) -> they run while ACT
                # loads the Exp table
                vector.wait_ge(sem_act1, 1)
                vector.tensor_scalar_add(
                    out=den[:], in0=normt[:], scalar1=EPS
                ).then_inc(sem_c4, 1)
                vector.wait_ge(sem_c4, 1)
                vector.reciprocal(out=rec[:], in_=den[:]).then_inc(sem_c4, 1)
                vector.wait_ge(sem_c4, 2)
                vector.tensor_mul(
                    out=ot[:].rearrange("b (d i) -> b d i", i=DD),
                    in0=s3, in1=rec[:].broadcast_to([BB, D, DD]),
                ).then_inc(sem_c4, 1)
                vector.wait_ge(sem_act2, 1)
                vector.tensor_scalar(
                    out=numt[:], in0=et[:], scalar1=-1.0, scalar2=1.0,
                    op0=mybir.AluOpType.mult, op1=mybir.AluOpType.add,
                ).then_inc(sem_c4, 1)
                vector.wait_ge(sem_c4, 4)
                o3 = ot[:].rearrange("b (d i) -> b d i", i=DD)
                vector.tensor_mul(
                    out=o3, in0=o3, in1=numt[:].broadcast_to([BB, D, DD]),
                ).then_inc(sem_fin, 1)

            @block.tensor
            def _(tensor):
                tensor.wait_ge(sem_u, 16)
                coff = 0
                for g, wsz in enumerate(WSIZES):
                    tensor.wait_ge(sem_dve, g + 1)
                    for k in range(wsz):
                        c = coff + k
                        mm = tensor.matmul(
                            ps[:],
                            lhsT=u_all[:, c * BB:(c + 1) * BB],
                            rhs=w_all[:, c * DI:(c + 1) * DI],
                            start=(c == 0), stop=(c == NCHUNK - 1),
                        )
                    coff += wsz
                mm.then_inc(sem_pe, 1)

            @block.scalar
            def _(scalar):
                # U^T on the ACT HWDGE ring (the SP ring is busy with W)
                scalar.dma_start(u_all[:], u_t[:, :]).then_inc(sem_u, 16)
                # ACT table warm-up.  The table RAM holds ONE function and
                # Copy/Sqrt share a table, so warming Sqrt covers both; Exp's
                # load is hidden behind DVE work.
                scalar.wait_ge(sem_wm, 1)
                scalar.activation(out=warm[:, 0:1], in_=warm[:, 1:2], func=AF.Sqrt)
                # epilogue: S copy (Copy shares the warmed Sqrt table)
                scalar.wait_ge(sem_pe, 1)
                scalar.activation(out=s[:], in_=ps[:], func=AF.Copy).then_inc(
                    sem_s1, 1
                )
                scalar.wait_ge(sem_v2, 1)
                scalar.activation(out=normt[:], in_=ss[:], func=AF.Sqrt).then_inc(
                    sem_act1, 1
                )
                scalar.wait_ge(sem_act1, 1)
                scalar.activation(
                    out=et[:], in_=normt[:], func=AF.Exp, scale=-1.0
                ).then_inc(sem_act2, 1)

    return nc


_CACHE = {}


def _get_nc():
    if "nc" not in _CACHE:
        _CACHE["nc"] = build_raw()
    return _CACHE["nc"]


def prep_inputs(primary_caps, W, B):
    """Host-side layout prep + sharding (no arithmetic).

    Contraction row order: chunk c holds n in [c*16, (c+1)*16); within a
    chunk, partition p = j*16 + n_local.
    """
    U = np.asarray(primary_caps, dtype=np.float32)
    Wnj = np.transpose(np.asarray(W, dtype=np.float32), (1, 3, 0, 2))  # n j d i
    # [c, p=(j,n_l), f] -> [p, (c f)]: one contiguous run per partition/group
    Wt = np.ascontiguousarray(
        Wnj.reshape(NCHUNK, 16, DP, DI)
        .transpose(0, 2, 1, 3)
        .reshape(NCHUNK, P, DI)
        .transpose(1, 0, 2)
        .reshape(P, NCHUNK * DI)
    )
    # bp[j*16+n_l, c*D+d] = B[d, c*16+n_l]  (replicated 8x across j)
    bp16 = (
        np.asarray(B, dtype=np.float32)
        .reshape(D, NCHUNK, 16)
        .transpose(2, 1, 0)
        .reshape(1, 16, NCHUNK * D)
    )
    bpm = np.ascontiguousarray(
        np.broadcast_to(bp16, (DP, 16, NCHUNK * D)).reshape(P, NCHUNK * D)
    )
    Unj = np.transpose(U, (1, 2, 0))  # n j b
    Ut = (
        Unj.reshape(NCHUNK, 16, DP, BFULL)
        .transpose(0, 2, 1, 3)
        .reshape(NCHUNK, P, BFULL)
        .transpose(1, 0, 2)  # [p, c, b]
    )
    return [
        {
            "u_t": np.ascontiguousarray(
                Ut[:, :, c * BB:(c + 1) * BB].reshape(P, NCHUNK * BB)
            ),
            "w_t": Wt,
            "bp": bpm,
        }
        for c in range(NCORES)
    ]


def kernel(primary_caps, W, B):
    nc = _get_nc()
    in_maps = prep_inputs(primary_caps, W, B)
    res = run_bass_kernel_spmd(nc, in_maps, core_ids=list(range(NCORES)))
    outs = [res.results[c]["out"] for c in range(NCORES)]
    return np.concatenate(outs, axis=0).reshape(BFULL, D, DD).astype(np.float32)
